# revision 18
# baseline (speedup 1.0000x reference)
"""Trainium2 Bass kernel for GNN message passing (nn_MessageModel).

Reference computation:
    inp = concat([x[col], edge_attr], 1)          # [E, 48]
    h = relu(inp @ W1 + b1)                       # [E, 64]
    messages = h @ W2 + b2                        # [E, 32]
    out = segment_sum(messages, row, N)           # [N, 32]

Strategy v3 (8 NeuronCores, SPMD, edge-pair stacking):
- Host: sort edges by destination row; pad every node's edge run to
  EVEN length with zero-input pad edges (message m0 = relu(b1)@W2,
  subtracted on host; m0 == 0 when b1 == 0).  Shard the padded edge
  sequence into 8 equal even-aligned chunks, cut into 512-edge lanes,
  8 lanes per block (2 scan-supergroups of 4 lanes).
- HW per block: DMA loads inpT [96, 2048] bf16 into SBUF partitions
  0:48 / 64:112.  W1 runs as 4-way-concurrent quadrant matmuls
  (tile 48x64) with even/odd strided rhs: even edges' h -> PSUM
  partitions 0:64, odd edges' h -> 64:128, per lane 256 pair-columns.
  ReLU+bias (ACT + DVE split) -> hS bf16.  W2 uses stacked weights
  [W2;W2] [128,32]: one matmul per lane sums each vertical pair ->
  msg per PAIR, [128 = 4 lanes x 32 feat, 256] -> msg2 [128, 512]
  per block.  DVE scan -> chained per-partition-lane cumsum over pair
  columns; GPSIMD ap_gather extracts 80 node-boundary columns per
  block; ext DMA out every 2 blocks.
- Host: per-chain adjacent differences, np.add.at merge of node parts
  (lanes/cores may split a node), pad-message correction, + deg * b2.
"""
import sys

if "/opt/trn_rl_repo" not in sys.path:
    sys.path.insert(0, "/opt/trn_rl_repo")

import numpy as np
import ml_dtypes

BF16 = ml_dtypes.bfloat16

N_NODES = 100000
N_EDGES = 1600000
D_NODE = 32
D_EDGE = 16
D_IN = D_NODE + D_EDGE
D_HID = 64
D_OUT = 32

N_CORES = 8
GE = 512                          # edge slots per lane
NP_ = GE // 2                     # pair slots per lane (256)
NS = 40                           # extraction slots per lane
NW = 2 * NS                       # idx per block window (80)
NWC = 8                           # idx cols reserved per window (16B aligned)
RELU_SPLIT = 320                  # cols of hS(sg1) relu done on DVE

_compiled_cache = {}


# ----------------------------------------------------------------------------
# host-side preprocessing
# ----------------------------------------------------------------------------

def _preprocess(x, edge_index, edge_attr, W1, b1, W2, b2):
    x = np.asarray(x, dtype=np.float32)
    W1 = np.asarray(W1, dtype=np.float32)
    b1 = np.asarray(b1, dtype=np.float32)
    W2 = np.asarray(W2, dtype=np.float32)
    b2 = np.asarray(b2, dtype=np.float32)
    row = np.asarray(edge_index[0], dtype=np.int64)
    col = np.asarray(edge_index[1], dtype=np.int64)
    order = np.argsort(row, kind="stable")
    row_s = row[order]
    col_s = col[order]
    attr_s = np.asarray(edge_attr, dtype=np.float32)[order]

    m0 = np.maximum(b1, 0.0).astype(np.float32) @ W2  # [32] pad-edge message

    # ---- pad each node's run to even length ----
    deg = np.bincount(row_s, minlength=N_NODES).astype(np.int64)
    dpad = deg + (deg & 1)
    # position of real edge e in the padded sequence
    off = np.zeros(N_NODES, dtype=np.int64)
    np.cumsum((deg & 1), out=off)          # inclusive; pads are AFTER runs
    off_excl = off - (deg & 1)             # pads before node n's run
    ppos = np.arange(N_EDGES) + off_excl[row_s]
    E_pad = int(N_EDGES + off[-1])
    # node id at every padded position (in-run pads carry the node id)
    rowp = np.repeat(np.arange(N_NODES), dpad)
    assert len(rowp) == E_pad

    # ---- core cut (even-aligned), lane/block geometry ----
    C = ((E_pad + 15) // 16) * 2           # even, C*8 >= E_pad
    n_lanes = ((-(-C // GE)) + 7) // 8 * 8
    slots_pc = n_lanes * GE
    n_blk = n_lanes // 8

    # ---- per-core slot arrays ----
    col_k = np.zeros((N_CORES, slots_pc), dtype=np.int64)
    attr_k = np.zeros((N_CORES, slots_pc, D_EDGE), dtype=np.float32)
    row_k = np.full((N_CORES, slots_pc), -1, dtype=np.int64)
    real_k = np.zeros((N_CORES, slots_pc), dtype=bool)
    colp = np.zeros(E_pad, dtype=np.int64)
    colp[ppos] = col_s
    attrp = np.zeros((E_pad, D_EDGE), dtype=np.float32)
    attrp[ppos] = attr_s
    realp = np.zeros(E_pad, dtype=bool)
    realp[ppos] = True
    for k in range(N_CORES):
        lo = k * C
        take = max(0, min(C, E_pad - lo))
        if take:
            col_k[k, :take] = colp[lo:lo + take]
            attr_k[k, :take] = attrp[lo:lo + take]
            row_k[k, :take] = rowp[lo:lo + take]
            real_k[k, :take] = realp[lo:lo + take]

    # ---- features & inpT tiles ----
    feat = np.zeros((N_CORES, slots_pc, D_IN), dtype=np.float32)
    feat[:, :, :D_NODE] = np.where(real_k[:, :, None], x[col_k], 0.0)
    feat[:, :, D_NODE:] = attr_k
    # [core, blk, sgi, pair, half, pp, two, f] -> [core, blk, 96, 2048]
    # col = 1024*sgi + 512*pair + 256*two + pp  (even/odd deinterleaved
    # on host so the W1 moving operand is contiguous)
    v = feat.reshape(N_CORES, n_blk, 2, 2, 2, NP_, 2, D_IN)
    inpT = np.ascontiguousarray(
        v.transpose(0, 1, 4, 7, 2, 3, 6, 5)
    ).reshape(N_CORES, n_blk, 2 * D_IN, 4 * GE).astype(BF16)

    # ---- node-end bookkeeping on padded slots ----
    pos_in_lane = np.arange(slots_pc) % GE
    lane_of = np.arange(slots_pc) // GE
    cores = []
    for k in range(N_CORES):
        re = row_k[k]
        valid = re >= 0
        flag = np.empty(slots_pc, dtype=bool)
        flag[:-1] = (re[:-1] != re[1:]) | (pos_in_lane[:-1] == GE - 1)
        flag[-1] = True
        flag &= valid
        ends = np.nonzero(flag)[0]
        lanes = lane_of[ends]
        pairp = pos_in_lane[ends] // 2
        nodes = re[ends]
        lane_start = np.searchsorted(lanes, np.arange(n_lanes), side="left")
        lane_end = np.searchsorted(lanes, np.arange(n_lanes), side="right")
        nn = lane_end - lane_start
        assert nn.max() <= NS, f"lane overflow: {nn.max()} > {NS}"
        si = np.arange(len(ends)) - lane_start[lanes]
        w = lanes // 8
        jj = lanes % 8
        sgi = jj // 4
        jslot = jj % 4                       # lane-in-sg by slot order
        # msg partition group (half-major, so concurrent same-bank W1
        # tiles share a row group): slot order is 2*pair+half, group is
        # 2*half+pair
        j = np.array([0, 2, 1, 3])[jslot]
        # chain-local pad count at each end (chain = slot-lane 8w+jslot of
        # sg0 then of sg1; the scan restarts per chain)
        padflag = (~real_k[k]).astype(np.int64)
        cumpad = np.cumsum(padflag)          # inclusive, per-core
        lane_base = lanes * GE
        cp_lane_excl = np.where(lane_base > 0, cumpad[lane_base - 1], 0)
        cp_in_lane = cumpad[ends] - cp_lane_excl
        lanepads = padflag.reshape(n_lanes, GE).sum(axis=1)
        chaincp = cp_in_lane + np.where(sgi == 1,
                                        lanepads[np.minimum(
                                            w * 8 + jslot, n_lanes - 1)], 0)
        cores.append(dict(ends=ends, pairp=pairp, nodes=nodes, si=si,
                          w=w, sgi=sgi, j=j, chaincp=chaincp))

    # ---- idx tiles [core, 128, n_blk*NWC] int16 ----
    idx_in = np.zeros((N_CORES, 128, n_blk * NWC), dtype=np.int16)
    for k in range(N_CORES):
        c = cores[k]
        i_flat = NS * c["sgi"] + c["si"]
        val = (c["pairp"] + NP_ * c["sgi"]).astype(np.int16)
        prow = 32 * c["j"] + (i_flat % 16)
        pcol = c["w"] * NWC + i_flat // 16
        idx_in[k, prow, pcol] = val
        idx_in[k, prow + 16, pcol] = val

    # ---- weights ----
    W1q = np.zeros((2 * D_IN, D_HID), dtype=BF16)  # rows 0:48 & 48:96 = W1
    W1q[:D_IN] = W1
    W1q[D_IN:] = W1
    W2sb = np.zeros((128, D_OUT), dtype=BF16)      # stacked [W2; W2]
    W2sb[:D_HID] = W2
    W2sb[D_HID:] = W2
    b1t = np.tile(b1[:, None], (2, 1)).astype(np.float32)

    return dict(cores=cores, inpT=inpT, idx_in=idx_in, n_blk=n_blk,
                W1q=W1q, W2sb=W2sb, b1t=b1t, m0=m0,
                deg=deg, b2=b2)


# ----------------------------------------------------------------------------
# numpy simulation of the HW dataflow (for correctness debugging)
# ----------------------------------------------------------------------------

def _simulate_hw(prep):
    n_blk = prep["n_blk"]
    W1f = prep["W1q"][:D_IN].astype(np.float32)    # [48, 64]
    W2f = prep["W2sb"].astype(np.float32)          # [128, 32]
    b1t = prep["b1t"][:, 0]
    ext_all = np.zeros((N_CORES, 128, n_blk * NW), dtype=np.float32)
    for k in range(N_CORES):
        for b in range(n_blk):
            inpT = prep["inpT"][k, b].astype(np.float32)
            msg2 = np.zeros((128, 2 * NP_), dtype=np.float32)
            for sgi in range(2):
                for pair in range(2):
                    for half in range(2):
                        g = 2 * half + pair
                        rhs = inpT[48 * half:48 * half + 48,
                                   1024 * sgi + 512 * pair:
                                   1024 * sgi + 512 * pair + 512]
                        hp = W1f.T @ rhs                    # [64, 512]
                        h = np.maximum(hp + b1t[:64, None], 0.0).astype(BF16)
                        h = h.astype(np.float32)
                        # cols 0:256 = even edges, 256:512 = odd (host
                        # deinterleaved); stack vertically
                        hpair = np.concatenate([h[:, :NP_], h[:, NP_:]], 0)
                        msg2[32 * g:32 * g + 32,
                             NP_ * sgi:NP_ * sgi + NP_] = W2f.T @ hpair
            cum = np.cumsum(msg2.astype(np.float64), axis=1).astype(np.float32)
            idxw = prep["idx_in"][k][:, b * NWC:b * NWC + NW // 16]
            for p in range(128):
                c16 = p // 16
                for i in range(NW):
                    ii = idxw[16 * c16 + (i % 16), i // 16]
                    ext_all[k, p, b * NW + i] = cum[p, ii]
    return ext_all


# ----------------------------------------------------------------------------
# assembly of the final output from extracted cumsums
# ----------------------------------------------------------------------------

def _assemble(prep, ext_all):
    out = np.zeros((N_NODES, D_OUT), dtype=np.float32)
    m0 = prep["m0"]
    use_m0 = bool(np.any(m0))
    for k in range(N_CORES):
        c = prep["cores"][k]
        nE = len(c["ends"])
        pcol = c["w"] * NW + NS * c["sgi"] + c["si"]
        prow = 32 * c["j"]
        V = np.empty((nE, D_OUT), dtype=np.float32)
        ek = ext_all[k]
        ar = np.arange(nE)
        for f in range(D_OUT):
            V[:, f] = ek[prow + f, pcol]
        key = ((c["w"] * 4 + c["j"]) * 2 + c["sgi"]) * (NS + 1) + c["si"]
        ordr = np.argsort(key, kind="stable")
        Vo = V[ordr]
        chain = (c["w"] * 4 + c["j"])[ordr]
        first = np.empty(nE, dtype=bool)
        first[0] = True
        first[1:] = chain[1:] != chain[:-1]
        diffs = Vo.copy()
        nf = np.nonzero(~first)[0]
        diffs[nf] -= Vo[nf - 1]
        if use_m0:
            cp = c["chaincp"][ordr].astype(np.float64)
            dp = cp.copy()
            dp[nf] -= cp[nf - 1]
            diffs -= (dp[:, None] * m0[None, :]).astype(np.float32)
        nodes_o = c["nodes"][ordr]
        np.add.at(out, nodes_o, diffs)
    out += prep["deg"][:, None] * prep["b2"][None, :]
    return out


# ----------------------------------------------------------------------------
# bass kernel
# ----------------------------------------------------------------------------

def _build_bass(n_blk):
    import concourse.bacc as bacc
    import concourse.mybir as mybir
    import concourse.tile as tile
    from contextlib import ExitStack

    nc = bacc.Bacc("TRN2", target_bir_lowering=False, debug=False,
                   enable_asserts=True, num_devices=N_CORES)
    f32 = mybir.dt.float32
    bf16 = mybir.dt.bfloat16
    inp_d = nc.dram_tensor("inpT", [n_blk, 2 * D_IN, 4 * GE], bf16,
                           kind="ExternalInput").ap()
    idx_d = nc.dram_tensor("idx", [128, n_blk * NWC], mybir.dt.int16,
                           kind="ExternalInput").ap()
    W1_d = nc.dram_tensor("W1q", [2 * D_IN, D_HID], bf16,
                          kind="ExternalInput").ap()
    W2_d = nc.dram_tensor("W2sb", [128, D_OUT], bf16,
                          kind="ExternalInput").ap()
    b1_d = nc.dram_tensor("b1t", [128, 1], f32, kind="ExternalInput").ap()
    ext_d = nc.dram_tensor("ext", [128, n_blk * NW], f32,
                           kind="ExternalOutput").ap()

    R = RELU_SPLIT

    with tile.TileContext(nc) as tc, ExitStack() as ctx:
        const = ctx.enter_context(tc.tile_pool(name="const", bufs=1))
        sb_in = ctx.enter_context(tc.tile_pool(name="sb_in", bufs=3))
        sb_h = ctx.enter_context(tc.tile_pool(name="sb_h", bufs=3))
        sb_out = ctx.enter_context(tc.tile_pool(name="sb_out", bufs=2))
        ps_h = ctx.enter_context(tc.tile_pool(name="ps_h", bufs=3,
                                              space="PSUM"))
        ps_m = ctx.enter_context(tc.tile_pool(name="ps_m", bufs=2,
                                              space="PSUM"))

        idx_all = const.tile([128, n_blk * NWC], mybir.dt.int16)
        nc.sync.dma_start(idx_all[:], idx_d[:])
        ones = const.tile([128, 2 * NP_], bf16)
        nc.gpsimd.memset(ones[:], 1.0)
        W1_s = const.tile([128, D_HID], bf16)
        nc.sync.dma_start(W1_s[0:48, :], W1_d[0:48])
        nc.sync.dma_start(W1_s[64:112, :], W1_d[48:96])
        W2_s = const.tile([128, D_OUT], bf16)
        nc.sync.dma_start(W2_s[:], W2_d[:])
        b1_s = const.tile([128, 1], f32)
        nc.sync.dma_start(b1_s[:], b1_d[:])

        inps, hps, hss, msgs, cums = {}, {}, {}, {}, {}
        ext_tiles = {}

        def emit_dma(b):
            t = sb_in.tile([128, 4 * GE], bf16, tag="inp", name=f"inp{b}")
            nc.sync.dma_start(t[0:48, :], inp_d[b][0:48])
            nc.sync.dma_start(t[64:112, :], inp_d[b][48:96])
            inps[b] = t

        def emit_w1(b, sgi):
            hS = ps_h.tile([128, 2 * GE], f32, tag="hS", name=f"hS{b}_{sgi}")
            t = inps[b]
            for pair in range(2):
                for half in range(2):
                    g = 2 * half + pair      # msg partition group / hS block
                    rb = 64 * half
                    cb = 1024 * sgi + 512 * pair
                    for eo in range(2):
                        nc.tensor.matmul(
                            hS[64 * eo:64 * eo + 64,
                               256 * g:256 * g + 256],
                            lhsT=W1_s[rb:rb + 48, :],
                            rhs=t[rb:rb + 48, cb + 256 * eo:cb + 256 * eo + 256],
                            start=True, stop=True,
                            tile_position=(64 * half, 64 * eo))
            hps[(b, sgi)] = hS

        def emit_relu(b, sgi):
            hS = hps[(b, sgi)]
            hSs = sb_h.tile([128, 2 * GE], bf16, tag="hSs",
                            name=f"hSs{b}_{sgi}")
            if sgi == 0:
                nc.scalar.activation(
                    out=hSs[:], in_=hS[:],
                    func=mybir.ActivationFunctionType.Relu, bias=b1_s[:])
            else:
                nc.vector.tensor_scalar(
                    out=hSs[:, 0:R], in0=hS[:, 0:R],
                    scalar1=b1_s[:], scalar2=0.0,
                    op0=mybir.AluOpType.add, op1=mybir.AluOpType.max)
                nc.scalar.activation(
                    out=hSs[:, R:], in_=hS[:, R:],
                    func=mybir.ActivationFunctionType.Relu, bias=b1_s[:])
            hss[(b, sgi)] = hSs

        def emit_w2(b):
            msg2 = ps_m.tile([128, 2 * NP_], f32, tag="msg", name=f"msg{b}")
            for sgi in range(2):
                hSs = hss[(b, sgi)]
                for jl in range(4):
                    nc.tensor.matmul(
                        msg2[32 * jl:32 * jl + 32,
                             NP_ * sgi:NP_ * sgi + NP_],
                        lhsT=W2_s[:], rhs=hSs[:, 256 * jl:256 * jl + 256],
                        start=True, stop=True, tile_position=(0, 32 * jl))
            msgs[b] = msg2

        def emit_scan(b):
            cum = sb_out.tile([128, 2 * NP_], f32, tag="cum", name=f"cum{b}")
            nc.vector.tensor_tensor_scan(
                out=cum[:], data0=ones[:], data1=msgs[b][:], initial=0.0,
                op0=mybir.AluOpType.mult, op1=mybir.AluOpType.add)
            cums[b] = cum

        def emit_gather(b):
            half = b % 2
            if half == 0:
                ext_s = sb_out.tile([128, 2 * NW], f32, tag="ext",
                                    name=f"ext{b}")
                ext_tiles[b] = ext_s
            ext_s = ext_tiles[b - half]
            nc.gpsimd.ap_gather(
                out_ap=ext_s[:, half * NW:(half + 1) * NW],
                in_ap=cums[b][:],
                idxs_ap=idx_all[:, b * NWC:b * NWC + NW // 16],
                channels=128, num_elems=2 * NP_, d=1, num_idxs=NW)
            if half == 1 or b == n_blk - 1:
                b0 = b - half
                nc.sync.dma_start(
                    ext_d[:, b0 * NW:(b + 1) * NW],
                    ext_s[:, :(half + 1) * NW])

        emit_dma(0)
        emit_dma(1)
        for b in range(n_blk):
            if b + 2 < n_blk:
                emit_dma(b + 2)
            emit_w1(b, 0)
            emit_relu(b, 0)
            emit_w1(b, 1)
            emit_relu(b, 1)
            if b > 0:
                emit_w2(b - 1)
                emit_scan(b - 1)
                emit_gather(b - 1)
        emit_w2(n_blk - 1)
        emit_scan(n_blk - 1)
        emit_gather(n_blk - 1)

    nc.compile()
    return nc


def _run_hw(prep, trace=False):
    from concourse.bass_utils import run_bass_kernel_spmd

    n_blk = prep["n_blk"]
    if n_blk not in _compiled_cache:
        _compiled_cache[n_blk] = _build_bass(n_blk)
    nc = _compiled_cache[n_blk]

    in_maps = []
    for k in range(N_CORES):
        in_maps.append({
            "inpT": prep["inpT"][k],
            "idx": prep["idx_in"][k],
            "W1q": prep["W1q"],
            "W2sb": prep["W2sb"],
            "b1t": prep["b1t"],
        })
    res = run_bass_kernel_spmd(nc, in_maps, list(range(N_CORES)), trace=trace)
    ext_all = np.stack([res.results[k]["ext"] for k in range(N_CORES)])
    return ext_all, res


def kernel(x, edge_index, edge_attr, W1, b1, W2, b2, _numpy_sim=False):
    prep = _preprocess(x, edge_index, edge_attr, W1, b1, W2, b2)
    if _numpy_sim:
        ext_all = _simulate_hw(prep)
    else:
        ext_all, _ = _run_hw(prep)
    return _assemble(prep, ext_all)


# revision 22
# speedup vs baseline: 1.0787x; 1.0787x over previous
"""Trainium2 Bass kernel for GNN message passing (nn_MessageModel).

Reference computation:
    inp = concat([x[col], edge_attr], 1)          # [E, 48]
    h = relu(inp @ W1 + b1)                       # [E, 64]
    messages = h @ W2 + b2                        # [E, 32]
    out = segment_sum(messages, row, N)           # [N, 32]

Strategy v3 (8 NeuronCores, SPMD, edge-pair stacking):
- Host: sort edges by destination row; pad every node's edge run to
  EVEN length with zero-input pad edges (message m0 = relu(b1)@W2,
  subtracted on host; m0 == 0 when b1 == 0).  Shard the padded edge
  sequence into 8 equal even-aligned chunks, cut into 512-edge lanes,
  8 lanes per block (2 scan-supergroups of 4 lanes).
- HW per block: DMA loads inpT [96, 2048] bf16 into SBUF partitions
  0:48 / 64:112.  W1 runs as 4-way-concurrent quadrant matmuls
  (tile 48x64) with even/odd strided rhs: even edges' h -> PSUM
  partitions 0:64, odd edges' h -> 64:128, per lane 256 pair-columns.
  ReLU+bias (ACT + DVE split) -> hS bf16.  W2 uses stacked weights
  [W2;W2] [128,32]: one matmul per lane sums each vertical pair ->
  msg per PAIR, [128 = 4 lanes x 32 feat, 256] -> msg2 [128, 512]
  per block.  DVE scan -> chained per-partition-lane cumsum over pair
  columns; GPSIMD ap_gather extracts 80 node-boundary columns per
  block; ext DMA out every 2 blocks.
- Host: per-chain adjacent differences, np.add.at merge of node parts
  (lanes/cores may split a node), pad-message correction, + deg * b2.
"""
import sys

if "/opt/trn_rl_repo" not in sys.path:
    sys.path.insert(0, "/opt/trn_rl_repo")

import numpy as np
import ml_dtypes

BF16 = ml_dtypes.bfloat16

N_NODES = 100000
N_EDGES = 1600000
D_NODE = 32
D_EDGE = 16
D_IN = D_NODE + D_EDGE
D_HID = 64
D_OUT = 32

N_CORES = 8
GE = 512                          # edge slots per lane
NP_ = GE // 2                     # pair slots per lane (256)
NS = 40                           # extraction slots per lane
NW = 2 * NS                       # idx per block window (80)
NWC = 8                           # idx cols reserved per window (16B aligned)
RELU_SPLIT = 320                  # cols of hS(sg1) relu done on DVE

_compiled_cache = {}


# ----------------------------------------------------------------------------
# host-side preprocessing
# ----------------------------------------------------------------------------

def _preprocess(x, edge_index, edge_attr, W1, b1, W2, b2):
    x = np.asarray(x, dtype=np.float32)
    W1 = np.asarray(W1, dtype=np.float32)
    b1 = np.asarray(b1, dtype=np.float32)
    W2 = np.asarray(W2, dtype=np.float32)
    b2 = np.asarray(b2, dtype=np.float32)
    row = np.asarray(edge_index[0], dtype=np.int64)
    col = np.asarray(edge_index[1], dtype=np.int64)
    order = np.argsort(row, kind="stable")
    row_s = row[order]
    col_s = col[order]
    attr_s = np.asarray(edge_attr, dtype=np.float32)[order]

    m0 = np.maximum(b1, 0.0).astype(np.float32) @ W2  # [32] pad-edge message

    # ---- pad each node's run to even length ----
    deg = np.bincount(row_s, minlength=N_NODES).astype(np.int64)
    dpad = deg + (deg & 1)
    # position of real edge e in the padded sequence
    off = np.zeros(N_NODES, dtype=np.int64)
    np.cumsum((deg & 1), out=off)          # inclusive; pads are AFTER runs
    off_excl = off - (deg & 1)             # pads before node n's run
    ppos = np.arange(N_EDGES) + off_excl[row_s]
    E_pad = int(N_EDGES + off[-1])
    # node id at every padded position (in-run pads carry the node id)
    rowp = np.repeat(np.arange(N_NODES), dpad)
    assert len(rowp) == E_pad

    # ---- core cut (even-aligned), lane/block geometry ----
    C = ((E_pad + 15) // 16) * 2           # even, C*8 >= E_pad
    n_lanes = ((-(-C // GE)) + 7) // 8 * 8
    slots_pc = n_lanes * GE
    n_blk = n_lanes // 8

    # ---- per-core slot arrays ----
    col_k = np.zeros((N_CORES, slots_pc), dtype=np.int64)
    attr_k = np.zeros((N_CORES, slots_pc, D_EDGE), dtype=np.float32)
    row_k = np.full((N_CORES, slots_pc), -1, dtype=np.int64)
    real_k = np.zeros((N_CORES, slots_pc), dtype=bool)
    colp = np.zeros(E_pad, dtype=np.int64)
    colp[ppos] = col_s
    attrp = np.zeros((E_pad, D_EDGE), dtype=np.float32)
    attrp[ppos] = attr_s
    realp = np.zeros(E_pad, dtype=bool)
    realp[ppos] = True
    for k in range(N_CORES):
        lo = k * C
        take = max(0, min(C, E_pad - lo))
        if take:
            col_k[k, :take] = colp[lo:lo + take]
            attr_k[k, :take] = attrp[lo:lo + take]
            row_k[k, :take] = rowp[lo:lo + take]
            real_k[k, :take] = realp[lo:lo + take]

    # ---- features & inpT tiles ----
    feat = np.zeros((N_CORES, slots_pc, D_IN), dtype=np.float32)
    feat[:, :, :D_NODE] = np.where(real_k[:, :, None], x[col_k], 0.0)
    feat[:, :, D_NODE:] = attr_k
    # [core, blk, sgi, pair, half, pp, two, f] -> [core, blk, 96, 2048]
    # col = 1024*sgi + 512*two + 256*pair + pp  (even/odd deinterleaved,
    # eo-major, so each W1 matmul reads a contiguous 512-col block
    # covering both pairs of one half)
    v = feat.reshape(N_CORES, n_blk, 2, 2, 2, NP_, 2, D_IN)
    inpT = np.ascontiguousarray(
        v.transpose(0, 1, 4, 7, 2, 6, 3, 5)
    ).reshape(N_CORES, n_blk, 2 * D_IN, 4 * GE).astype(BF16)

    # ---- node-end bookkeeping on padded slots ----
    pos_in_lane = np.arange(slots_pc) % GE
    lane_of = np.arange(slots_pc) // GE
    cores = []
    for k in range(N_CORES):
        re = row_k[k]
        valid = re >= 0
        flag = np.empty(slots_pc, dtype=bool)
        flag[:-1] = (re[:-1] != re[1:]) | (pos_in_lane[:-1] == GE - 1)
        flag[-1] = True
        flag &= valid
        ends = np.nonzero(flag)[0]
        lanes = lane_of[ends]
        pairp = pos_in_lane[ends] // 2
        nodes = re[ends]
        lane_start = np.searchsorted(lanes, np.arange(n_lanes), side="left")
        lane_end = np.searchsorted(lanes, np.arange(n_lanes), side="right")
        nn = lane_end - lane_start
        assert nn.max() <= NS, f"lane overflow: {nn.max()} > {NS}"
        si = np.arange(len(ends)) - lane_start[lanes]
        w = lanes // 8
        jj = lanes % 8
        sgi = jj // 4
        jslot = jj % 4                       # lane-in-sg by slot order
        # msg partition group (half-major, so concurrent same-bank W1
        # tiles share a row group): slot order is 2*pair+half, group is
        # 2*half+pair
        j = np.array([0, 2, 1, 3])[jslot]
        # chain-local pad count at each end (chain = slot-lane 8w+jslot of
        # sg0 then of sg1; the scan restarts per chain)
        padflag = (~real_k[k]).astype(np.int64)
        cumpad = np.cumsum(padflag)          # inclusive, per-core
        lane_base = lanes * GE
        cp_lane_excl = np.where(lane_base > 0, cumpad[lane_base - 1], 0)
        cp_in_lane = cumpad[ends] - cp_lane_excl
        lanepads = padflag.reshape(n_lanes, GE).sum(axis=1)
        chaincp = cp_in_lane + np.where(sgi == 1,
                                        lanepads[np.minimum(
                                            w * 8 + jslot, n_lanes - 1)], 0)
        cores.append(dict(ends=ends, pairp=pairp, nodes=nodes, si=si,
                          w=w, sgi=sgi, j=j, chaincp=chaincp))

    # ---- idx tiles [core, 128, n_blk*NWC] int16 ----
    idx_in = np.zeros((N_CORES, 128, n_blk * NWC), dtype=np.int16)
    for k in range(N_CORES):
        c = cores[k]
        i_flat = NS * c["sgi"] + c["si"]
        val = (c["pairp"] + NP_ * c["sgi"]).astype(np.int16)
        prow = 32 * c["j"] + (i_flat % 16)
        pcol = c["w"] * NWC + i_flat // 16
        idx_in[k, prow, pcol] = val
        idx_in[k, prow + 16, pcol] = val

    # ---- weights ----
    W1q = np.zeros((2 * D_IN, D_HID), dtype=BF16)  # rows 0:48 & 48:96 = W1
    W1q[:D_IN] = W1
    W1q[D_IN:] = W1
    W2sb = np.zeros((128, D_OUT), dtype=BF16)      # stacked [W2; W2]
    W2sb[:D_HID] = W2
    W2sb[D_HID:] = W2
    b1t = np.tile(b1[:, None], (2, 1)).astype(np.float32)

    return dict(cores=cores, inpT=inpT, idx_in=idx_in, n_blk=n_blk,
                W1q=W1q, W2sb=W2sb, b1t=b1t, m0=m0,
                deg=deg, b2=b2)


# ----------------------------------------------------------------------------
# numpy simulation of the HW dataflow (for correctness debugging)
# ----------------------------------------------------------------------------

def _simulate_hw(prep):
    n_blk = prep["n_blk"]
    W1f = prep["W1q"][:D_IN].astype(np.float32)    # [48, 64]
    W2f = prep["W2sb"].astype(np.float32)          # [128, 32]
    b1t = prep["b1t"][:, 0]
    ext_all = np.zeros((N_CORES, 128, n_blk * NW), dtype=np.float32)
    for k in range(N_CORES):
        for b in range(n_blk):
            inpT = prep["inpT"][k, b].astype(np.float32)
            msg2 = np.zeros((128, 2 * NP_), dtype=np.float32)
            for sgi in range(2):
                for half in range(2):
                    # eo-major: cols [1024*sgi + 512*eo + 256*pair + pp]
                    he = W1f.T @ inpT[48 * half:48 * half + 48,
                                      1024 * sgi:1024 * sgi + 512]
                    ho = W1f.T @ inpT[48 * half:48 * half + 48,
                                      1024 * sgi + 512:1024 * sgi + 1024]
                    he = np.maximum(he + b1t[:64, None], 0.0).astype(BF16)
                    ho = np.maximum(ho + b1t[:64, None], 0.0).astype(BF16)
                    he = he.astype(np.float32)
                    ho = ho.astype(np.float32)
                    for pair in range(2):
                        g = 2 * half + pair
                        hpair = np.concatenate(
                            [he[:, 256 * pair:256 * pair + 256],
                             ho[:, 256 * pair:256 * pair + 256]], 0)
                        msg2[32 * g:32 * g + 32,
                             NP_ * sgi:NP_ * sgi + NP_] = W2f.T @ hpair
            cum = np.cumsum(msg2.astype(np.float64), axis=1).astype(np.float32)
            idxw = prep["idx_in"][k][:, b * NWC:b * NWC + NW // 16]
            for p in range(128):
                c16 = p // 16
                for i in range(NW):
                    ii = idxw[16 * c16 + (i % 16), i // 16]
                    ext_all[k, p, b * NW + i] = cum[p, ii]
    return ext_all


# ----------------------------------------------------------------------------
# assembly of the final output from extracted cumsums
# ----------------------------------------------------------------------------

def _assemble(prep, ext_all):
    out = np.zeros((N_NODES, D_OUT), dtype=np.float32)
    m0 = prep["m0"]
    use_m0 = bool(np.any(m0))
    for k in range(N_CORES):
        c = prep["cores"][k]
        nE = len(c["ends"])
        pcol = c["w"] * NW + NS * c["sgi"] + c["si"]
        prow = 32 * c["j"]
        V = np.empty((nE, D_OUT), dtype=np.float32)
        ek = ext_all[k]
        ar = np.arange(nE)
        for f in range(D_OUT):
            V[:, f] = ek[prow + f, pcol]
        key = ((c["w"] * 4 + c["j"]) * 2 + c["sgi"]) * (NS + 1) + c["si"]
        ordr = np.argsort(key, kind="stable")
        Vo = V[ordr]
        chain = (c["w"] * 4 + c["j"])[ordr]
        first = np.empty(nE, dtype=bool)
        first[0] = True
        first[1:] = chain[1:] != chain[:-1]
        diffs = Vo.copy()
        nf = np.nonzero(~first)[0]
        diffs[nf] -= Vo[nf - 1]
        if use_m0:
            cp = c["chaincp"][ordr].astype(np.float64)
            dp = cp.copy()
            dp[nf] -= cp[nf - 1]
            diffs -= (dp[:, None] * m0[None, :]).astype(np.float32)
        nodes_o = c["nodes"][ordr]
        np.add.at(out, nodes_o, diffs)
    out += prep["deg"][:, None] * prep["b2"][None, :]
    return out


# ----------------------------------------------------------------------------
# bass kernel
# ----------------------------------------------------------------------------

def _build_bass(n_blk):
    import concourse.bacc as bacc
    import concourse.mybir as mybir
    import concourse.tile as tile
    from contextlib import ExitStack

    nc = bacc.Bacc("TRN2", target_bir_lowering=False, debug=False,
                   enable_asserts=True, num_devices=N_CORES)
    f32 = mybir.dt.float32
    bf16 = mybir.dt.bfloat16
    inp_d = nc.dram_tensor("inpT", [n_blk, 2 * D_IN, 4 * GE], bf16,
                           kind="ExternalInput").ap()
    idx_d = nc.dram_tensor("idx", [128, n_blk * NWC], mybir.dt.int16,
                           kind="ExternalInput").ap()
    W1_d = nc.dram_tensor("W1q", [2 * D_IN, D_HID], bf16,
                          kind="ExternalInput").ap()
    W2_d = nc.dram_tensor("W2sb", [128, D_OUT], bf16,
                          kind="ExternalInput").ap()
    b1_d = nc.dram_tensor("b1t", [128, 1], f32, kind="ExternalInput").ap()
    ext_d = nc.dram_tensor("ext", [128, n_blk * NW], f32,
                           kind="ExternalOutput").ap()

    R = RELU_SPLIT

    with tile.TileContext(nc) as tc, ExitStack() as ctx:
        const = ctx.enter_context(tc.tile_pool(name="const", bufs=1))
        sb_in = ctx.enter_context(tc.tile_pool(name="sb_in", bufs=3))
        sb_h = ctx.enter_context(tc.tile_pool(name="sb_h", bufs=3))
        sb_out = ctx.enter_context(tc.tile_pool(name="sb_out", bufs=2))
        ps_h = ctx.enter_context(tc.tile_pool(name="ps_h", bufs=3,
                                              space="PSUM"))
        ps_m = ctx.enter_context(tc.tile_pool(name="ps_m", bufs=2,
                                              space="PSUM"))

        idx_all = const.tile([128, n_blk * NWC], mybir.dt.int16)
        nc.sync.dma_start(idx_all[:], idx_d[:])
        ones = const.tile([128, 2 * NP_], bf16)
        nc.gpsimd.memset(ones[:], 1.0)
        W1_s = const.tile([128, D_HID], bf16)
        nc.sync.dma_start(W1_s[0:48, :], W1_d[0:48])
        nc.sync.dma_start(W1_s[64:112, :], W1_d[48:96])
        W2_s = const.tile([128, D_OUT], bf16)
        nc.sync.dma_start(W2_s[:], W2_d[:])
        b1_s = const.tile([128, 1], f32)
        nc.sync.dma_start(b1_s[:], b1_d[:])

        inps, hps, hss, msgs, cums = {}, {}, {}, {}, {}
        ext_tiles = {}

        def emit_dma(b):
            t = sb_in.tile([128, 4 * GE], bf16, tag="inp", name=f"inp{b}")
            nc.sync.dma_start(t[0:48, :], inp_d[b][0:48])
            nc.sync.dma_start(t[64:112, :], inp_d[b][48:96])
            inps[b] = t

        def emit_w1(b, sgi):
            # hS cols: [512*eo + 256*pair + pp]; one 512-col matmul per
            # (half, eo) quadrant tile covers both pairs
            hS = ps_h.tile([128, 2 * GE], f32, tag="hS", name=f"hS{b}_{sgi}")
            t = inps[b]
            for half in range(2):
                rb = 64 * half
                for eo in range(2):
                    nc.tensor.matmul(
                        hS[64 * eo:64 * eo + 64, 512 * half:512 * half + 512],
                        lhsT=W1_s[rb:rb + 48, :],
                        rhs=t[rb:rb + 48,
                              1024 * sgi + 512 * eo:1024 * sgi + 512 * eo + 512],
                        start=True, stop=True,
                        tile_position=(64 * half, 64 * eo))
            hps[(b, sgi)] = hS

        def emit_relu(b, sgi):
            # hS (PSUM, per sg): [64*eo + hid, 512*half + 256*pair + pp]
            # hSb (SBUF, per block): cols [512*g + 256*sgi + pp], g=2h+p
            hS = hps[(b, sgi)]
            if sgi == 0:
                hSb = sb_h.tile([128, 4 * GE], bf16, tag="hSb",
                                name=f"hSb{b}")
                hss[b] = hSb
            hSb = hss[b]
            h4 = hSb.rearrange("p (g s c) -> p g s c", g=4, s=2)
            # hS flat col = 256*g + pp (g = 2*half+pair); hSb col =
            # 512*g + 256*sgi + pp, so the out view h4[:, :, sgi, :]
            # iterates in the same (g, pp) order as hS.
            if sgi == 0:
                nc.scalar.activation(
                    out=h4[:, :, 0:1, :], in_=hS[:],
                    func=mybir.ActivationFunctionType.Relu, bias=b1_s[:])
            else:
                nc.vector.tensor_scalar(
                    out=hSb[:, NP_:2 * NP_], in0=hS[:, 0:NP_],
                    scalar1=b1_s[:], scalar2=0.0,
                    op0=mybir.AluOpType.add, op1=mybir.AluOpType.max)
                nc.scalar.activation(
                    out=h4[:, 1:4, 1:2, :], in_=hS[:, NP_:],
                    func=mybir.ActivationFunctionType.Relu, bias=b1_s[:])

        def emit_w2(b):
            msg2 = ps_m.tile([128, 2 * NP_], f32, tag="msg", name=f"msg{b}")
            hSb = hss[b]
            for g in range(4):
                nc.tensor.matmul(
                    msg2[32 * g:32 * g + 32, :],
                    lhsT=W2_s[:], rhs=hSb[:, 512 * g:512 * g + 512],
                    start=True, stop=True, tile_position=(0, 32 * g))
            msgs[b] = msg2

        def emit_scan(b):
            cum = sb_out.tile([128, 2 * NP_], f32, tag="cum", name=f"cum{b}")
            nc.vector.tensor_tensor_scan(
                out=cum[:], data0=ones[:], data1=msgs[b][:], initial=0.0,
                op0=mybir.AluOpType.mult, op1=mybir.AluOpType.add)
            cums[b] = cum

        def emit_gather(b):
            half = b % 2
            if half == 0:
                ext_s = sb_out.tile([128, 2 * NW], f32, tag="ext",
                                    name=f"ext{b}")
                ext_tiles[b] = ext_s
            ext_s = ext_tiles[b - half]
            nc.gpsimd.ap_gather(
                out_ap=ext_s[:, half * NW:(half + 1) * NW],
                in_ap=cums[b][:],
                idxs_ap=idx_all[:, b * NWC:b * NWC + NW // 16],
                channels=128, num_elems=2 * NP_, d=1, num_idxs=NW)
            if half == 1 or b == n_blk - 1:
                b0 = b - half
                nc.sync.dma_start(
                    ext_d[:, b0 * NW:(b + 1) * NW],
                    ext_s[:, :(half + 1) * NW])

        emit_dma(0)
        emit_dma(1)
        for b in range(n_blk):
            if b + 2 < n_blk:
                emit_dma(b + 2)
            emit_w1(b, 0)
            emit_relu(b, 0)
            emit_w1(b, 1)
            emit_relu(b, 1)
            if b > 0:
                emit_w2(b - 1)
                emit_scan(b - 1)
                emit_gather(b - 1)
        emit_w2(n_blk - 1)
        emit_scan(n_blk - 1)
        emit_gather(n_blk - 1)

    nc.compile()
    return nc


def _run_hw(prep, trace=False):
    from concourse.bass_utils import run_bass_kernel_spmd

    n_blk = prep["n_blk"]
    if n_blk not in _compiled_cache:
        _compiled_cache[n_blk] = _build_bass(n_blk)
    nc = _compiled_cache[n_blk]

    in_maps = []
    for k in range(N_CORES):
        in_maps.append({
            "inpT": prep["inpT"][k],
            "idx": prep["idx_in"][k],
            "W1q": prep["W1q"],
            "W2sb": prep["W2sb"],
            "b1t": prep["b1t"],
        })
    res = run_bass_kernel_spmd(nc, in_maps, list(range(N_CORES)), trace=trace)
    ext_all = np.stack([res.results[k]["ext"] for k in range(N_CORES)])
    return ext_all, res


def kernel(x, edge_index, edge_attr, W1, b1, W2, b2, _numpy_sim=False):
    prep = _preprocess(x, edge_index, edge_attr, W1, b1, W2, b2)
    if _numpy_sim:
        ext_all = _simulate_hw(prep)
    else:
        ext_all, _ = _run_hw(prep)
    return _assemble(prep, ext_all)


# revision 28
# speedup vs baseline: 1.1364x; 1.0534x over previous
"""Trainium2 Bass kernel for GNN message passing (nn_MessageModel).

Reference computation:
    inp = concat([x[col], edge_attr], 1)          # [E, 48]
    h = relu(inp @ W1 + b1)                       # [E, 64]
    messages = h @ W2 + b2                        # [E, 32]
    out = segment_sum(messages, row, N)           # [N, 32]

Strategy v3 (8 NeuronCores, SPMD, edge-pair stacking):
- Host: sort edges by destination row; pad every node's edge run to
  EVEN length with zero-input pad edges (message m0 = relu(b1)@W2,
  subtracted on host; m0 == 0 when b1 == 0).  Shard the padded edge
  sequence into 8 equal even-aligned chunks, cut into 512-edge lanes,
  8 lanes per block (2 scan-supergroups of 4 lanes).
- HW per block: DMA loads inpT [96, 2048] bf16 into SBUF partitions
  0:48 / 64:112.  W1 runs as 4-way-concurrent quadrant matmuls
  (tile 48x64) with even/odd strided rhs: even edges' h -> PSUM
  partitions 0:64, odd edges' h -> 64:128, per lane 256 pair-columns.
  ReLU+bias (ACT + DVE split) -> hS bf16.  W2 uses stacked weights
  [W2;W2] [128,32]: one matmul per lane sums each vertical pair ->
  msg per PAIR, [128 = 4 lanes x 32 feat, 256] -> msg2 [128, 512]
  per block.  DVE scan -> chained per-partition-lane cumsum over pair
  columns; GPSIMD ap_gather extracts 80 node-boundary columns per
  block; ext DMA out every 2 blocks.
- Host: per-chain adjacent differences, np.add.at merge of node parts
  (lanes/cores may split a node), pad-message correction, + deg * b2.
"""
import sys

if "/opt/trn_rl_repo" not in sys.path:
    sys.path.insert(0, "/opt/trn_rl_repo")

import numpy as np
import ml_dtypes

BF16 = ml_dtypes.bfloat16

N_NODES = 100000
N_EDGES = 1600000
D_NODE = 32
D_EDGE = 16
D_IN = D_NODE + D_EDGE
D_HID = 64
D_OUT = 32

N_CORES = 8
GE = 512                          # edge slots per lane
NP_ = GE // 2                     # pair slots per lane (256)
NS = 40                           # extraction slots per lane
NW = 2 * NS                       # idx per block window (80)
NWC = 8                           # idx cols reserved per window (16B aligned)
RELU_SPLIT = 128                  # relu cols per sg done on DVE
GRP = 4                           # blocks per phase-batched group

_compiled_cache = {}


# ----------------------------------------------------------------------------
# host-side preprocessing
# ----------------------------------------------------------------------------

def _preprocess(x, edge_index, edge_attr, W1, b1, W2, b2):
    x = np.asarray(x, dtype=np.float32)
    W1 = np.asarray(W1, dtype=np.float32)
    b1 = np.asarray(b1, dtype=np.float32)
    W2 = np.asarray(W2, dtype=np.float32)
    b2 = np.asarray(b2, dtype=np.float32)
    row = np.asarray(edge_index[0], dtype=np.int64)
    col = np.asarray(edge_index[1], dtype=np.int64)
    order = np.argsort(row, kind="stable")
    row_s = row[order]
    col_s = col[order]
    attr_s = np.asarray(edge_attr, dtype=np.float32)[order]

    m0 = np.maximum(b1, 0.0).astype(np.float32) @ W2  # [32] pad-edge message

    # ---- pad each node's run to even length ----
    deg = np.bincount(row_s, minlength=N_NODES).astype(np.int64)
    dpad = deg + (deg & 1)
    # position of real edge e in the padded sequence
    off = np.zeros(N_NODES, dtype=np.int64)
    np.cumsum((deg & 1), out=off)          # inclusive; pads are AFTER runs
    off_excl = off - (deg & 1)             # pads before node n's run
    ppos = np.arange(N_EDGES) + off_excl[row_s]
    E_pad = int(N_EDGES + off[-1])
    # node id at every padded position (in-run pads carry the node id)
    rowp = np.repeat(np.arange(N_NODES), dpad)
    assert len(rowp) == E_pad

    # ---- core cut (even-aligned), lane/block geometry ----
    C = ((E_pad + 15) // 16) * 2           # even, C*8 >= E_pad
    n_lanes = ((-(-C // GE)) + 7) // 8 * 8
    slots_pc = n_lanes * GE
    n_blk = n_lanes // 8

    # ---- per-core slot arrays ----
    col_k = np.zeros((N_CORES, slots_pc), dtype=np.int64)
    attr_k = np.zeros((N_CORES, slots_pc, D_EDGE), dtype=np.float32)
    row_k = np.full((N_CORES, slots_pc), -1, dtype=np.int64)
    real_k = np.zeros((N_CORES, slots_pc), dtype=bool)
    colp = np.zeros(E_pad, dtype=np.int64)
    colp[ppos] = col_s
    attrp = np.zeros((E_pad, D_EDGE), dtype=np.float32)
    attrp[ppos] = attr_s
    realp = np.zeros(E_pad, dtype=bool)
    realp[ppos] = True
    for k in range(N_CORES):
        lo = k * C
        take = max(0, min(C, E_pad - lo))
        if take:
            col_k[k, :take] = colp[lo:lo + take]
            attr_k[k, :take] = attrp[lo:lo + take]
            row_k[k, :take] = rowp[lo:lo + take]
            real_k[k, :take] = realp[lo:lo + take]

    # ---- features & inpT tiles ----
    feat = np.zeros((N_CORES, slots_pc, D_IN), dtype=np.float32)
    feat[:, :, :D_NODE] = np.where(real_k[:, :, None], x[col_k], 0.0)
    feat[:, :, D_NODE:] = attr_k
    # [core, blk, sgi, pair, half, pp, two, f] -> [core, blk, 96, 2048]
    # col = 1024*sgi + 512*two + 256*pair + pp  (even/odd deinterleaved,
    # eo-major, so each W1 matmul reads a contiguous 512-col block
    # covering both pairs of one half)
    v = feat.reshape(N_CORES, n_blk, 2, 2, 2, NP_, 2, D_IN)
    inpT = np.ascontiguousarray(
        v.transpose(0, 1, 4, 7, 2, 6, 3, 5)
    ).reshape(N_CORES, n_blk, 2 * D_IN, 4 * GE).astype(BF16)

    # ---- node-end bookkeeping on padded slots ----
    pos_in_lane = np.arange(slots_pc) % GE
    lane_of = np.arange(slots_pc) // GE
    cores = []
    for k in range(N_CORES):
        re = row_k[k]
        valid = re >= 0
        flag = np.empty(slots_pc, dtype=bool)
        flag[:-1] = (re[:-1] != re[1:]) | (pos_in_lane[:-1] == GE - 1)
        flag[-1] = True
        flag &= valid
        ends = np.nonzero(flag)[0]
        lanes = lane_of[ends]
        pairp = pos_in_lane[ends] // 2
        nodes = re[ends]
        lane_start = np.searchsorted(lanes, np.arange(n_lanes), side="left")
        lane_end = np.searchsorted(lanes, np.arange(n_lanes), side="right")
        nn = lane_end - lane_start
        assert nn.max() <= NS, f"lane overflow: {nn.max()} > {NS}"
        si = np.arange(len(ends)) - lane_start[lanes]
        w = lanes // 8
        jj = lanes % 8
        sgi = jj // 4
        jslot = jj % 4                       # lane-in-sg by slot order
        # msg partition group (half-major, so concurrent same-bank W1
        # tiles share a row group): slot order is 2*pair+half, group is
        # 2*half+pair
        j = np.array([0, 2, 1, 3])[jslot]
        # chain-local pad count at each end (chain = slot-lane 8w+jslot of
        # sg0 then of sg1; the scan restarts per chain)
        padflag = (~real_k[k]).astype(np.int64)
        cumpad = np.cumsum(padflag)          # inclusive, per-core
        lane_base = lanes * GE
        cp_lane_excl = np.where(lane_base > 0, cumpad[lane_base - 1], 0)
        cp_in_lane = cumpad[ends] - cp_lane_excl
        lanepads = padflag.reshape(n_lanes, GE).sum(axis=1)
        chaincp = cp_in_lane + np.where(sgi == 1,
                                        lanepads[np.minimum(
                                            w * 8 + jslot, n_lanes - 1)], 0)
        cores.append(dict(ends=ends, pairp=pairp, nodes=nodes, si=si,
                          w=w, sgi=sgi, j=j, chaincp=chaincp))

    # ---- idx tiles [core, 128, n_blk*NWC] int16 ----
    idx_in = np.zeros((N_CORES, 128, n_blk * NWC), dtype=np.int16)
    for k in range(N_CORES):
        c = cores[k]
        i_flat = NS * c["sgi"] + c["si"]
        val = (c["pairp"] + NP_ * c["sgi"]).astype(np.int16)
        prow = 32 * c["j"] + (i_flat % 16)
        pcol = c["w"] * NWC + i_flat // 16
        idx_in[k, prow, pcol] = val
        idx_in[k, prow + 16, pcol] = val

    # ---- weights ----
    W1q = np.zeros((2 * D_IN, D_HID), dtype=BF16)  # rows 0:48 & 48:96 = W1
    W1q[:D_IN] = W1
    W1q[D_IN:] = W1
    W2sb = np.zeros((128, D_OUT), dtype=BF16)      # stacked [W2; W2]
    W2sb[:D_HID] = W2
    W2sb[D_HID:] = W2
    b1t = np.tile(b1[:, None], (2, 1)).astype(np.float32)

    return dict(cores=cores, inpT=inpT, idx_in=idx_in, n_blk=n_blk,
                W1q=W1q, W2sb=W2sb, b1t=b1t, m0=m0,
                deg=deg, b2=b2)


# ----------------------------------------------------------------------------
# numpy simulation of the HW dataflow (for correctness debugging)
# ----------------------------------------------------------------------------

def _simulate_hw(prep):
    n_blk = prep["n_blk"]
    W1f = prep["W1q"][:D_IN].astype(np.float32)    # [48, 64]
    W2f = prep["W2sb"].astype(np.float32)          # [128, 32]
    b1t = prep["b1t"][:, 0]
    ext_all = np.zeros((N_CORES, 128, n_blk * NW), dtype=np.float32)
    for k in range(N_CORES):
        for b in range(n_blk):
            inpT = prep["inpT"][k, b].astype(np.float32)
            msg2 = np.zeros((128, 2 * NP_), dtype=np.float32)
            for sgi in range(2):
                for half in range(2):
                    # eo-major: cols [1024*sgi + 512*eo + 256*pair + pp]
                    he = W1f.T @ inpT[48 * half:48 * half + 48,
                                      1024 * sgi:1024 * sgi + 512]
                    ho = W1f.T @ inpT[48 * half:48 * half + 48,
                                      1024 * sgi + 512:1024 * sgi + 1024]
                    he = np.maximum(he + b1t[:64, None], 0.0).astype(BF16)
                    ho = np.maximum(ho + b1t[:64, None], 0.0).astype(BF16)
                    he = he.astype(np.float32)
                    ho = ho.astype(np.float32)
                    for pair in range(2):
                        g = 2 * half + pair
                        hpair = np.concatenate(
                            [he[:, 256 * pair:256 * pair + 256],
                             ho[:, 256 * pair:256 * pair + 256]], 0)
                        msg2[32 * g:32 * g + 32,
                             NP_ * sgi:NP_ * sgi + NP_] = W2f.T @ hpair
            cum = np.cumsum(msg2.astype(np.float64), axis=1).astype(np.float32)
            idxw = prep["idx_in"][k][:, b * NWC:b * NWC + NW // 16]
            for p in range(128):
                c16 = p // 16
                for i in range(NW):
                    ii = idxw[16 * c16 + (i % 16), i // 16]
                    ext_all[k, p, b * NW + i] = cum[p, ii]
    return ext_all


# ----------------------------------------------------------------------------
# assembly of the final output from extracted cumsums
# ----------------------------------------------------------------------------

def _assemble(prep, ext_all):
    out = np.zeros((N_NODES, D_OUT), dtype=np.float32)
    m0 = prep["m0"]
    use_m0 = bool(np.any(m0))
    for k in range(N_CORES):
        c = prep["cores"][k]
        nE = len(c["ends"])
        pcol = c["w"] * NW + NS * c["sgi"] + c["si"]
        prow = 32 * c["j"]
        V = np.empty((nE, D_OUT), dtype=np.float32)
        ek = ext_all[k]
        ar = np.arange(nE)
        for f in range(D_OUT):
            V[:, f] = ek[prow + f, pcol]
        key = ((c["w"] * 4 + c["j"]) * 2 + c["sgi"]) * (NS + 1) + c["si"]
        ordr = np.argsort(key, kind="stable")
        Vo = V[ordr]
        chain = (c["w"] * 4 + c["j"])[ordr]
        first = np.empty(nE, dtype=bool)
        first[0] = True
        first[1:] = chain[1:] != chain[:-1]
        diffs = Vo.copy()
        nf = np.nonzero(~first)[0]
        diffs[nf] -= Vo[nf - 1]
        if use_m0:
            cp = c["chaincp"][ordr].astype(np.float64)
            dp = cp.copy()
            dp[nf] -= cp[nf - 1]
            diffs -= (dp[:, None] * m0[None, :]).astype(np.float32)
        nodes_o = c["nodes"][ordr]
        np.add.at(out, nodes_o, diffs)
    out += prep["deg"][:, None] * prep["b2"][None, :]
    return out


# ----------------------------------------------------------------------------
# bass kernel
# ----------------------------------------------------------------------------

def _build_bass(n_blk):
    import concourse.bacc as bacc
    import concourse.mybir as mybir
    import concourse.tile as tile
    from contextlib import ExitStack

    nc = bacc.Bacc("TRN2", target_bir_lowering=False, debug=False,
                   enable_asserts=True, num_devices=N_CORES)
    f32 = mybir.dt.float32
    bf16 = mybir.dt.bfloat16
    inp_d = nc.dram_tensor("inpT", [n_blk, 2 * D_IN, 4 * GE], bf16,
                           kind="ExternalInput").ap()
    idx_d = nc.dram_tensor("idx", [128, n_blk * NWC], mybir.dt.int16,
                           kind="ExternalInput").ap()
    W1_d = nc.dram_tensor("W1q", [2 * D_IN, D_HID], bf16,
                          kind="ExternalInput").ap()
    W2_d = nc.dram_tensor("W2sb", [128, D_OUT], bf16,
                          kind="ExternalInput").ap()
    b1_d = nc.dram_tensor("b1t", [128, 1], f32, kind="ExternalInput").ap()
    ext_d = nc.dram_tensor("ext", [128, n_blk * NW], f32,
                           kind="ExternalOutput").ap()

    RD = RELU_SPLIT

    with tile.TileContext(nc) as tc, ExitStack() as ctx:
        const = ctx.enter_context(tc.tile_pool(name="const", bufs=1))
        sb_in = ctx.enter_context(tc.tile_pool(name="sb_in", bufs=GRP + 4))
        sb_h = ctx.enter_context(tc.tile_pool(name="sb_h", bufs=18))
        sb_out = ctx.enter_context(tc.tile_pool(name="sb_out", bufs=3))
        ps_h = ctx.enter_context(tc.tile_pool(name="ps_h", bufs=3,
                                              space="PSUM"))
        ps_m = ctx.enter_context(tc.tile_pool(name="ps_m", bufs=2,
                                              space="PSUM"))

        idx_all = const.tile([128, n_blk * NWC], mybir.dt.int16)
        nc.sync.dma_start(idx_all[:], idx_d[:])
        ones = const.tile([128, 2 * NP_], bf16)
        nc.gpsimd.memset(ones[:], 1.0)
        W1_s = const.tile([128, D_HID], bf16)
        nc.sync.dma_start(W1_s[0:48, :], W1_d[0:48])
        nc.sync.dma_start(W1_s[64:112, :], W1_d[48:96])
        W2_s = const.tile([128, D_OUT], bf16)
        nc.sync.dma_start(W2_s[:], W2_d[:])
        b1_s = const.tile([128, 1], f32)
        nc.sync.dma_start(b1_s[:], b1_d[:])

        inps, hps, hss, msgs, cums = {}, {}, {}, {}, {}
        ext_tiles = {}

        def emit_dma(b):
            t = sb_in.tile([128, 4 * GE], bf16, tag="inp", name=f"inp{b}")
            nc.sync.dma_start(t[0:48, :], inp_d[b][0:48])
            nc.sync.dma_start(t[64:112, :], inp_d[b][48:96])
            inps[b] = t

        def emit_w1(b, sgi):
            # hS cols: [512*eo + 256*pair + pp]; one 512-col matmul per
            # (half, eo) quadrant tile covers both pairs
            hS = ps_h.tile([128, 2 * GE], f32, tag="hS", name=f"hS{b}_{sgi}")
            t = inps[b]
            for half in range(2):
                rb = 64 * half
                for eo in range(2):
                    nc.tensor.matmul(
                        hS[64 * eo:64 * eo + 64, 512 * half:512 * half + 512],
                        lhsT=W1_s[rb:rb + 48, :],
                        rhs=t[rb:rb + 48,
                              1024 * sgi + 512 * eo:1024 * sgi + 512 * eo + 512],
                        start=True, stop=True,
                        tile_position=(64 * half, 64 * eo))
            hps[(b, sgi)] = hS

        def emit_relu(b, sgi):
            # hS (PSUM) and hSs (SBUF) share the g-major column layout
            # [256*g + pp], so the relu is a 2D contiguous copy.  DVE
            # takes the last RD columns, ACT the rest.
            hS = hps[(b, sgi)]
            hSs = sb_h.tile([128, 2 * GE], bf16, tag="hSs",
                            name=f"hSs{b}_{sgi}")
            nc.scalar.activation(
                out=hSs[:, 0:2 * GE - RD], in_=hS[:, 0:2 * GE - RD],
                func=mybir.ActivationFunctionType.Relu, bias=b1_s[:])
            nc.vector.tensor_scalar(
                out=hSs[:, 2 * GE - RD:], in0=hS[:, 2 * GE - RD:],
                scalar1=b1_s[:], scalar2=0.0,
                op0=mybir.AluOpType.add, op1=mybir.AluOpType.max)
            hss[(b, sgi)] = hSs

        def emit_w2(b):
            msg2 = ps_m.tile([128, 2 * NP_], f32, tag="msg", name=f"msg{b}")
            for sgi in range(2):
                hSs = hss[(b, sgi)]
                for g in range(4):
                    nc.tensor.matmul(
                        msg2[32 * g:32 * g + 32,
                             NP_ * sgi:NP_ * sgi + NP_],
                        lhsT=W2_s[:], rhs=hSs[:, 256 * g:256 * g + 256],
                        start=True, stop=True, tile_position=(0, 32 * g))
            msgs[b] = msg2

        def emit_scan(b):
            cum = sb_out.tile([128, 2 * NP_], f32, tag="cum", name=f"cum{b}")
            nc.vector.tensor_tensor_scan(
                out=cum[:], data0=ones[:], data1=msgs[b][:], initial=0.0,
                op0=mybir.AluOpType.mult, op1=mybir.AluOpType.add)
            cums[b] = cum

        def emit_gather(b):
            half = b % 2
            if half == 0:
                ext_s = sb_out.tile([128, 2 * NW], f32, tag="ext",
                                    name=f"ext{b}")
                ext_tiles[b] = ext_s
            ext_s = ext_tiles[b - half]
            nc.gpsimd.ap_gather(
                out_ap=ext_s[:, half * NW:(half + 1) * NW],
                in_ap=cums[b][:],
                idxs_ap=idx_all[:, b * NWC:b * NWC + NW // 16],
                channels=128, num_elems=2 * NP_, d=1, num_idxs=NW)
            if half == 1 or b == n_blk - 1:
                b0 = b - half
                nc.sync.dma_start(
                    ext_d[:, b0 * NW:(b + 1) * NW],
                    ext_s[:, :(half + 1) * NW])

        # phase-batched groups: per group emit all W1+relu, then the
        # PREVIOUS group's W2/scan/gather (keeps PE on one weight set
        # for a whole phase; W2's relu inputs are long since ready)
        groups = [list(range(g, min(g + GRP, n_blk)))
                  for g in range(0, n_blk, GRP)]
        for b in range(min(GRP + 2, n_blk)):
            emit_dma(b)
        prev = []
        for grp in groups:
            for b in grp:
                if b + GRP + 2 < n_blk:
                    emit_dma(b + GRP + 2)
                emit_w1(b, 0)
                emit_relu(b, 0)
                emit_w1(b, 1)
                emit_relu(b, 1)
            for b in prev:
                emit_w2(b)
            for b in prev:
                emit_scan(b)
                emit_gather(b)
            prev = grp
        for b in prev:
            emit_w2(b)
        for b in prev:
            emit_scan(b)
            emit_gather(b)

    nc.compile()
    return nc


def _run_hw(prep, trace=False):
    from concourse.bass_utils import run_bass_kernel_spmd

    n_blk = prep["n_blk"]
    if n_blk not in _compiled_cache:
        _compiled_cache[n_blk] = _build_bass(n_blk)
    nc = _compiled_cache[n_blk]

    in_maps = []
    for k in range(N_CORES):
        in_maps.append({
            "inpT": prep["inpT"][k],
            "idx": prep["idx_in"][k],
            "W1q": prep["W1q"],
            "W2sb": prep["W2sb"],
            "b1t": prep["b1t"],
        })
    res = run_bass_kernel_spmd(nc, in_maps, list(range(N_CORES)), trace=trace)
    ext_all = np.stack([res.results[k]["ext"] for k in range(N_CORES)])
    return ext_all, res


def kernel(x, edge_index, edge_attr, W1, b1, W2, b2, _numpy_sim=False):
    prep = _preprocess(x, edge_index, edge_attr, W1, b1, W2, b2)
    if _numpy_sim:
        ext_all = _simulate_hw(prep)
    else:
        ext_all, _ = _run_hw(prep)
    return _assemble(prep, ext_all)


# revision 29
# speedup vs baseline: 1.1430x; 1.0058x over previous
"""Trainium2 Bass kernel for GNN message passing (nn_MessageModel).

Reference computation:
    inp = concat([x[col], edge_attr], 1)          # [E, 48]
    h = relu(inp @ W1 + b1)                       # [E, 64]
    messages = h @ W2 + b2                        # [E, 32]
    out = segment_sum(messages, row, N)           # [N, 32]

Strategy v3 (8 NeuronCores, SPMD, edge-pair stacking):
- Host: sort edges by destination row; pad every node's edge run to
  EVEN length with zero-input pad edges (message m0 = relu(b1)@W2,
  subtracted on host; m0 == 0 when b1 == 0).  Shard the padded edge
  sequence into 8 equal even-aligned chunks, cut into 512-edge lanes,
  8 lanes per block (2 scan-supergroups of 4 lanes).
- HW per block: DMA loads inpT [96, 2048] bf16 into SBUF partitions
  0:48 / 64:112.  W1 runs as 4-way-concurrent quadrant matmuls
  (tile 48x64) with even/odd strided rhs: even edges' h -> PSUM
  partitions 0:64, odd edges' h -> 64:128, per lane 256 pair-columns.
  ReLU+bias (ACT + DVE split) -> hS bf16.  W2 uses stacked weights
  [W2;W2] [128,32]: one matmul per lane sums each vertical pair ->
  msg per PAIR, [128 = 4 lanes x 32 feat, 256] -> msg2 [128, 512]
  per block.  DVE scan -> chained per-partition-lane cumsum over pair
  columns; GPSIMD ap_gather extracts 80 node-boundary columns per
  block; ext DMA out every 2 blocks.
- Host: per-chain adjacent differences, np.add.at merge of node parts
  (lanes/cores may split a node), pad-message correction, + deg * b2.
"""
import sys

if "/opt/trn_rl_repo" not in sys.path:
    sys.path.insert(0, "/opt/trn_rl_repo")

import numpy as np
import ml_dtypes

BF16 = ml_dtypes.bfloat16

N_NODES = 100000
N_EDGES = 1600000
D_NODE = 32
D_EDGE = 16
D_IN = D_NODE + D_EDGE
D_HID = 64
D_OUT = 32

N_CORES = 8
GE = 512                          # edge slots per lane
NP_ = GE // 2                     # pair slots per lane (256)
NS = 40                           # extraction slots per lane
NW = 2 * NS                       # idx per block window (80)
NWC = 8                           # idx cols reserved per window (16B aligned)
RELU_SPLIT = 128                  # relu cols per sg done on DVE
GRP = 4                           # blocks per phase-batched group

_compiled_cache = {}


# ----------------------------------------------------------------------------
# host-side preprocessing
# ----------------------------------------------------------------------------

def _preprocess(x, edge_index, edge_attr, W1, b1, W2, b2):
    x = np.asarray(x, dtype=np.float32)
    W1 = np.asarray(W1, dtype=np.float32)
    b1 = np.asarray(b1, dtype=np.float32)
    W2 = np.asarray(W2, dtype=np.float32)
    b2 = np.asarray(b2, dtype=np.float32)
    row = np.asarray(edge_index[0], dtype=np.int64)
    col = np.asarray(edge_index[1], dtype=np.int64)
    order = np.argsort(row, kind="stable")
    row_s = row[order]
    col_s = col[order]
    attr_s = np.asarray(edge_attr, dtype=np.float32)[order]

    m0 = np.maximum(b1, 0.0).astype(np.float32) @ W2  # [32] pad-edge message

    # ---- pad each node's run to even length ----
    deg = np.bincount(row_s, minlength=N_NODES).astype(np.int64)
    dpad = deg + (deg & 1)
    # position of real edge e in the padded sequence
    off = np.zeros(N_NODES, dtype=np.int64)
    np.cumsum((deg & 1), out=off)          # inclusive; pads are AFTER runs
    off_excl = off - (deg & 1)             # pads before node n's run
    ppos = np.arange(N_EDGES) + off_excl[row_s]
    E_pad = int(N_EDGES + off[-1])
    # node id at every padded position (in-run pads carry the node id)
    rowp = np.repeat(np.arange(N_NODES), dpad)
    assert len(rowp) == E_pad

    # ---- core cut (even-aligned), lane/block geometry ----
    C = ((E_pad + 15) // 16) * 2           # even, C*8 >= E_pad
    n_lanes = ((-(-C // GE)) + 7) // 8 * 8
    slots_pc = n_lanes * GE
    n_blk = n_lanes // 8

    # ---- per-core slot arrays ----
    col_k = np.zeros((N_CORES, slots_pc), dtype=np.int64)
    attr_k = np.zeros((N_CORES, slots_pc, D_EDGE), dtype=np.float32)
    row_k = np.full((N_CORES, slots_pc), -1, dtype=np.int64)
    real_k = np.zeros((N_CORES, slots_pc), dtype=bool)
    colp = np.zeros(E_pad, dtype=np.int64)
    colp[ppos] = col_s
    attrp = np.zeros((E_pad, D_EDGE), dtype=np.float32)
    attrp[ppos] = attr_s
    realp = np.zeros(E_pad, dtype=bool)
    realp[ppos] = True
    for k in range(N_CORES):
        lo = k * C
        take = max(0, min(C, E_pad - lo))
        if take:
            col_k[k, :take] = colp[lo:lo + take]
            attr_k[k, :take] = attrp[lo:lo + take]
            row_k[k, :take] = rowp[lo:lo + take]
            real_k[k, :take] = realp[lo:lo + take]

    # ---- features & inpT tiles ----
    feat = np.zeros((N_CORES, slots_pc, D_IN), dtype=np.float32)
    feat[:, :, :D_NODE] = np.where(real_k[:, :, None], x[col_k], 0.0)
    feat[:, :, D_NODE:] = attr_k
    # [core, blk, sgi, pair, half, pp, two, f] -> [core, blk, 96, 2048]
    # col = 1024*sgi + 512*two + 256*pair + pp  (even/odd deinterleaved,
    # eo-major, so each W1 matmul reads a contiguous 512-col block
    # covering both pairs of one half)
    v = feat.reshape(N_CORES, n_blk, 2, 2, 2, NP_, 2, D_IN)
    inpT = np.ascontiguousarray(
        v.transpose(0, 1, 4, 7, 2, 6, 3, 5)
    ).reshape(N_CORES, n_blk, 2 * D_IN, 4 * GE).astype(BF16)

    # ---- node-end bookkeeping on padded slots ----
    pos_in_lane = np.arange(slots_pc) % GE
    lane_of = np.arange(slots_pc) // GE
    cores = []
    for k in range(N_CORES):
        re = row_k[k]
        valid = re >= 0
        flag = np.empty(slots_pc, dtype=bool)
        flag[:-1] = (re[:-1] != re[1:]) | (pos_in_lane[:-1] == GE - 1)
        flag[-1] = True
        flag &= valid
        ends = np.nonzero(flag)[0]
        lanes = lane_of[ends]
        pairp = pos_in_lane[ends] // 2
        nodes = re[ends]
        lane_start = np.searchsorted(lanes, np.arange(n_lanes), side="left")
        lane_end = np.searchsorted(lanes, np.arange(n_lanes), side="right")
        nn = lane_end - lane_start
        assert nn.max() <= NS, f"lane overflow: {nn.max()} > {NS}"
        si = np.arange(len(ends)) - lane_start[lanes]
        w = lanes // 8
        jj = lanes % 8
        sgi = jj // 4
        jslot = jj % 4                       # lane-in-sg by slot order
        # msg partition group (half-major, so concurrent same-bank W1
        # tiles share a row group): slot order is 2*pair+half, group is
        # 2*half+pair
        j = np.array([0, 2, 1, 3])[jslot]
        # chain-local pad count at each end (chain = slot-lane 8w+jslot of
        # sg0 then of sg1; the scan restarts per chain)
        padflag = (~real_k[k]).astype(np.int64)
        cumpad = np.cumsum(padflag)          # inclusive, per-core
        lane_base = lanes * GE
        cp_lane_excl = np.where(lane_base > 0, cumpad[lane_base - 1], 0)
        cp_in_lane = cumpad[ends] - cp_lane_excl
        lanepads = padflag.reshape(n_lanes, GE).sum(axis=1)
        chaincp = cp_in_lane + np.where(sgi == 1,
                                        lanepads[np.minimum(
                                            w * 8 + jslot, n_lanes - 1)], 0)
        cores.append(dict(ends=ends, pairp=pairp, nodes=nodes, si=si,
                          w=w, sgi=sgi, j=j, chaincp=chaincp))

    # ---- idx tiles [core, 128, n_blk*NWC] int16 ----
    idx_in = np.zeros((N_CORES, 128, n_blk * NWC), dtype=np.int16)
    for k in range(N_CORES):
        c = cores[k]
        i_flat = NS * c["sgi"] + c["si"]
        val = (c["pairp"] + NP_ * c["sgi"]).astype(np.int16)
        prow = 32 * c["j"] + (i_flat % 16)
        pcol = c["w"] * NWC + i_flat // 16
        idx_in[k, prow, pcol] = val
        idx_in[k, prow + 16, pcol] = val

    # ---- weights ----
    W1q = np.zeros((2 * D_IN, D_HID), dtype=BF16)  # rows 0:48 & 48:96 = W1
    W1q[:D_IN] = W1
    W1q[D_IN:] = W1
    W2sb = np.zeros((128, D_OUT), dtype=BF16)      # stacked [W2; W2]
    W2sb[:D_HID] = W2
    W2sb[D_HID:] = W2
    b1t = np.tile(b1[:, None], (2, 1)).astype(np.float32)

    return dict(cores=cores, inpT=inpT, idx_in=idx_in, n_blk=n_blk,
                W1q=W1q, W2sb=W2sb, b1t=b1t, m0=m0,
                deg=deg, b2=b2)


# ----------------------------------------------------------------------------
# numpy simulation of the HW dataflow (for correctness debugging)
# ----------------------------------------------------------------------------

def _simulate_hw(prep):
    n_blk = prep["n_blk"]
    W1f = prep["W1q"][:D_IN].astype(np.float32)    # [48, 64]
    W2f = prep["W2sb"].astype(np.float32)          # [128, 32]
    b1t = prep["b1t"][:, 0]
    ext_all = np.zeros((N_CORES, 128, n_blk * NW), dtype=np.float32)
    for k in range(N_CORES):
        for b in range(n_blk):
            inpT = prep["inpT"][k, b].astype(np.float32)
            msg2 = np.zeros((128, 2 * NP_), dtype=np.float32)
            for sgi in range(2):
                for half in range(2):
                    # eo-major: cols [1024*sgi + 512*eo + 256*pair + pp]
                    he = W1f.T @ inpT[48 * half:48 * half + 48,
                                      1024 * sgi:1024 * sgi + 512]
                    ho = W1f.T @ inpT[48 * half:48 * half + 48,
                                      1024 * sgi + 512:1024 * sgi + 1024]
                    he = np.maximum(he + b1t[:64, None], 0.0).astype(BF16)
                    ho = np.maximum(ho + b1t[:64, None], 0.0).astype(BF16)
                    he = he.astype(np.float32)
                    ho = ho.astype(np.float32)
                    for pair in range(2):
                        g = 2 * half + pair
                        hpair = np.concatenate(
                            [he[:, 256 * pair:256 * pair + 256],
                             ho[:, 256 * pair:256 * pair + 256]], 0)
                        msg2[32 * g:32 * g + 32,
                             NP_ * sgi:NP_ * sgi + NP_] = W2f.T @ hpair
            cum = np.cumsum(msg2.astype(np.float64), axis=1).astype(np.float32)
            idxw = prep["idx_in"][k][:, b * NWC:b * NWC + NW // 16]
            for p in range(128):
                c16 = p // 16
                for i in range(NW):
                    ii = idxw[16 * c16 + (i % 16), i // 16]
                    ext_all[k, p, b * NW + i] = cum[p, ii]
    return ext_all


# ----------------------------------------------------------------------------
# assembly of the final output from extracted cumsums
# ----------------------------------------------------------------------------

def _assemble(prep, ext_all):
    out = np.zeros((N_NODES, D_OUT), dtype=np.float32)
    m0 = prep["m0"]
    use_m0 = bool(np.any(m0))
    for k in range(N_CORES):
        c = prep["cores"][k]
        nE = len(c["ends"])
        pcol = c["w"] * NW + NS * c["sgi"] + c["si"]
        prow = 32 * c["j"]
        V = np.empty((nE, D_OUT), dtype=np.float32)
        ek = ext_all[k]
        ar = np.arange(nE)
        for f in range(D_OUT):
            V[:, f] = ek[prow + f, pcol]
        key = ((c["w"] * 4 + c["j"]) * 2 + c["sgi"]) * (NS + 1) + c["si"]
        ordr = np.argsort(key, kind="stable")
        Vo = V[ordr]
        chain = (c["w"] * 4 + c["j"])[ordr]
        first = np.empty(nE, dtype=bool)
        first[0] = True
        first[1:] = chain[1:] != chain[:-1]
        diffs = Vo.copy()
        nf = np.nonzero(~first)[0]
        diffs[nf] -= Vo[nf - 1]
        if use_m0:
            cp = c["chaincp"][ordr].astype(np.float64)
            dp = cp.copy()
            dp[nf] -= cp[nf - 1]
            diffs -= (dp[:, None] * m0[None, :]).astype(np.float32)
        nodes_o = c["nodes"][ordr]
        np.add.at(out, nodes_o, diffs)
    out += prep["deg"][:, None] * prep["b2"][None, :]
    return out


# ----------------------------------------------------------------------------
# bass kernel
# ----------------------------------------------------------------------------

def _build_bass(n_blk):
    import concourse.bacc as bacc
    import concourse.mybir as mybir
    import concourse.tile as tile
    from contextlib import ExitStack

    nc = bacc.Bacc("TRN2", target_bir_lowering=False, debug=False,
                   enable_asserts=True, num_devices=N_CORES)
    f32 = mybir.dt.float32
    bf16 = mybir.dt.bfloat16
    inp_d = nc.dram_tensor("inpT", [n_blk, 2 * D_IN, 4 * GE], bf16,
                           kind="ExternalInput").ap()
    idx_d = nc.dram_tensor("idx", [128, n_blk * NWC], mybir.dt.int16,
                           kind="ExternalInput").ap()
    W1_d = nc.dram_tensor("W1q", [2 * D_IN, D_HID], bf16,
                          kind="ExternalInput").ap()
    W2_d = nc.dram_tensor("W2sb", [128, D_OUT], bf16,
                          kind="ExternalInput").ap()
    b1_d = nc.dram_tensor("b1t", [128, 1], f32, kind="ExternalInput").ap()
    ext_d = nc.dram_tensor("ext", [128, n_blk * NW], f32,
                           kind="ExternalOutput").ap()

    RD = RELU_SPLIT

    with tile.TileContext(nc) as tc, ExitStack() as ctx:
        const = ctx.enter_context(tc.tile_pool(name="const", bufs=1))
        sb_in = ctx.enter_context(tc.tile_pool(name="sb_in", bufs=GRP + 4))
        sb_h = ctx.enter_context(tc.tile_pool(name="sb_h", bufs=18))
        sb_out = ctx.enter_context(tc.tile_pool(name="sb_out", bufs=3))
        ps_h = ctx.enter_context(tc.tile_pool(name="ps_h", bufs=3,
                                              space="PSUM"))
        ps_m = ctx.enter_context(tc.tile_pool(name="ps_m", bufs=2,
                                              space="PSUM"))

        idx_all = const.tile([128, n_blk * NWC], mybir.dt.int16)
        nc.sync.dma_start(idx_all[:], idx_d[:])
        ones = const.tile([128, 2 * NP_], bf16)
        nc.gpsimd.memset(ones[:], 1.0)
        W1_s = const.tile([128, D_HID], bf16)
        nc.sync.dma_start(W1_s[0:48, :], W1_d[0:48])
        nc.sync.dma_start(W1_s[64:112, :], W1_d[48:96])
        W2_s = const.tile([128, D_OUT], bf16)
        nc.sync.dma_start(W2_s[:], W2_d[:])
        b1_s = const.tile([128, 1], f32)
        nc.sync.dma_start(b1_s[:], b1_d[:])

        inps, hps, hss, msgs, cums = {}, {}, {}, {}, {}
        ext_tiles = {}

        def emit_dma(b):
            t = sb_in.tile([128, 4 * GE], bf16, tag="inp", name=f"inp{b}")
            nc.sync.dma_start(t[0:48, :], inp_d[b][0:48])
            nc.sync.dma_start(t[64:112, :], inp_d[b][48:96])
            inps[b] = t

        def emit_w1(b, sgi):
            # hS cols: [512*eo + 256*pair + pp]; one 512-col matmul per
            # (half, eo) quadrant tile covers both pairs
            hS = ps_h.tile([128, 2 * GE], f32, tag="hS", name=f"hS{b}_{sgi}")
            t = inps[b]
            # eo-major order alternates PE row groups (0/64) so each
            # LDWEIGHTS overlaps the other row group's in-flight matmul
            for eo in range(2):
                for half in range(2):
                    rb = 64 * half
                    nc.tensor.matmul(
                        hS[64 * eo:64 * eo + 64, 512 * half:512 * half + 512],
                        lhsT=W1_s[rb:rb + 48, :],
                        rhs=t[rb:rb + 48,
                              1024 * sgi + 512 * eo:1024 * sgi + 512 * eo + 512],
                        start=True, stop=True,
                        tile_position=(64 * half, 64 * eo))
            hps[(b, sgi)] = hS

        def emit_relu(b, sgi):
            # hS (PSUM) and hSs (SBUF) share the g-major column layout
            # [256*g + pp], so the relu is a 2D contiguous copy.  DVE
            # takes the last RD columns, ACT the rest.
            hS = hps[(b, sgi)]
            hSs = sb_h.tile([128, 2 * GE], bf16, tag="hSs",
                            name=f"hSs{b}_{sgi}")
            nc.scalar.activation(
                out=hSs[:, 0:2 * GE - RD], in_=hS[:, 0:2 * GE - RD],
                func=mybir.ActivationFunctionType.Relu, bias=b1_s[:])
            nc.vector.tensor_scalar(
                out=hSs[:, 2 * GE - RD:], in0=hS[:, 2 * GE - RD:],
                scalar1=b1_s[:], scalar2=0.0,
                op0=mybir.AluOpType.add, op1=mybir.AluOpType.max)
            hss[(b, sgi)] = hSs

        def emit_w2(b):
            msg2 = ps_m.tile([128, 2 * NP_], f32, tag="msg", name=f"msg{b}")
            for sgi in range(2):
                hSs = hss[(b, sgi)]
                for g in range(4):
                    nc.tensor.matmul(
                        msg2[32 * g:32 * g + 32,
                             NP_ * sgi:NP_ * sgi + NP_],
                        lhsT=W2_s[:], rhs=hSs[:, 256 * g:256 * g + 256],
                        start=True, stop=True, tile_position=(0, 32 * g))
            msgs[b] = msg2

        def emit_scan(b):
            cum = sb_out.tile([128, 2 * NP_], f32, tag="cum", name=f"cum{b}")
            nc.vector.tensor_tensor_scan(
                out=cum[:], data0=ones[:], data1=msgs[b][:], initial=0.0,
                op0=mybir.AluOpType.mult, op1=mybir.AluOpType.add)
            cums[b] = cum

        def emit_gather(b):
            half = b % 2
            if half == 0:
                ext_s = sb_out.tile([128, 2 * NW], f32, tag="ext",
                                    name=f"ext{b}")
                ext_tiles[b] = ext_s
            ext_s = ext_tiles[b - half]
            nc.gpsimd.ap_gather(
                out_ap=ext_s[:, half * NW:(half + 1) * NW],
                in_ap=cums[b][:],
                idxs_ap=idx_all[:, b * NWC:b * NWC + NW // 16],
                channels=128, num_elems=2 * NP_, d=1, num_idxs=NW)
            if half == 1 or b == n_blk - 1:
                b0 = b - half
                nc.sync.dma_start(
                    ext_d[:, b0 * NW:(b + 1) * NW],
                    ext_s[:, :(half + 1) * NW])

        # phase-batched groups: per group emit all W1+relu, then the
        # PREVIOUS group's W2/scan/gather (keeps PE on one weight set
        # for a whole phase; W2's relu inputs are long since ready)
        groups = [list(range(g, min(g + GRP, n_blk)))
                  for g in range(0, n_blk, GRP)]
        for b in range(min(GRP + 2, n_blk)):
            emit_dma(b)
        prev = []
        for grp in groups:
            for b in grp:
                if b + GRP + 2 < n_blk:
                    emit_dma(b + GRP + 2)
                emit_w1(b, 0)
                emit_relu(b, 0)
                emit_w1(b, 1)
                emit_relu(b, 1)
            for b in prev:
                emit_w2(b)
            for b in prev:
                emit_scan(b)
                emit_gather(b)
            prev = grp
        for b in prev:
            emit_w2(b)
        for b in prev:
            emit_scan(b)
            emit_gather(b)

    nc.compile()
    return nc


def _run_hw(prep, trace=False):
    from concourse.bass_utils import run_bass_kernel_spmd

    n_blk = prep["n_blk"]
    if n_blk not in _compiled_cache:
        _compiled_cache[n_blk] = _build_bass(n_blk)
    nc = _compiled_cache[n_blk]

    in_maps = []
    for k in range(N_CORES):
        in_maps.append({
            "inpT": prep["inpT"][k],
            "idx": prep["idx_in"][k],
            "W1q": prep["W1q"],
            "W2sb": prep["W2sb"],
            "b1t": prep["b1t"],
        })
    res = run_bass_kernel_spmd(nc, in_maps, list(range(N_CORES)), trace=trace)
    ext_all = np.stack([res.results[k]["ext"] for k in range(N_CORES)])
    return ext_all, res


def kernel(x, edge_index, edge_attr, W1, b1, W2, b2, _numpy_sim=False):
    prep = _preprocess(x, edge_index, edge_attr, W1, b1, W2, b2)
    if _numpy_sim:
        ext_all = _simulate_hw(prep)
    else:
        ext_all, _ = _run_hw(prep)
    return _assemble(prep, ext_all)
